# revision 1
# baseline (speedup 1.0000x reference)
"""AWBNet (wo R2) Trainium2 kernel.

Math (per sample b):
  m = reshape(relu(hist_flat @ W1 + b1) @ W2 + b2, [9, 3])
  feats(px) = [r, g, b, r^2, g^2, b^2, rg, rb, gb]
  y[px, c] = sum_k feats[px, k] * m[k, c]

Device strategy (8 cores, pure data parallel over batch, 2 samples/core):
  * Tiny MLP on TensorE in fp32 with natural layouts (host only re-packs
    histogram / b1 so no on-device transposes are needed).
  * Per-pixel einsum on VectorE/ScalarE in fp16 using the square basis
    {R, G, B, R^2, G^2, B^2, (R+G)^2, (R+B)^2, (G+B)^2}; the coefficient
    change (rg = ((R+G)^2 - R^2 - G^2)/2 etc.) is folded into W2/b2 on the
    host (pure linear re-parameterization of the weights, no data compute).
  * m-coefficients are broadcast to all 128 partitions by a fused
    matmul whose lhsT is a stride-0 (broadcast) column of featT; they are
    then per-partition scalars for the per-pixel products.
  * Per-pixel: ScalarE deinterleaves (stride-3 fp32 -> dense fp16) and
    squares; products m_k*F_k split DVE tensor_scalar (2x) / ACT
    scale-copies to balance the engines; DVE tt-add trees combine, the
    last add writing the stride-3 fp32 output view directly.
  * Three DMA queues in parallel: W1 stream + late x tiles on SWDGE
    (with fp32->fp16 cast), x0/x1 + y stores on the SP HWDGE ring, small
    setup DMAs on the ACT HWDGE ring.
"""

import sys

import numpy as np

for _p in ("/opt/trn_rl_repo",):
    if _p not in sys.path:
        sys.path.insert(0, _p)

import concourse.bacc as bacc
import concourse.mybir as mybir
import concourse.tile as tile
from concourse import bass_utils

# ---- problem constants (hardcoded per contract) ----
N_CORES = 8
B, H, W, C = 16, 512, 512, 3
SPC = B // N_CORES  # samples per core = 2
PX_SAMPLE = H * W  # 262144
PX_CORE = SPC * PX_SAMPLE  # 524288
P = 128
LANE_PX = PX_CORE // P  # 4096 pixels per partition per core
T = 1024  # pixels per partition per tile
NTILES = LANE_PX // T  # 4
TILES_PER_SAMPLE = NTILES // SPC  # 2

HIST = 3 * 64 * 64  # 12288
HID = 256
MOUT = 27
KT = HIST // P  # 96 k-tiles
MT = HID // P  # 2 m-tiles
W1_CH = 8  # k-tiles per W1 DMA chunk (8 * 128KB = 1MB)
KT_SH = KT // N_CORES  # 12 k-tiles of W1 per core (K-sharded MLP + AllReduce)

F16 = mybir.dt.float16
BF16 = mybir.dt.bfloat16
PLDT = mybir.dt.float16
F32 = mybir.dt.float32
MULT = mybir.AluOpType.mult
ADD = mybir.AluOpType.add
AF = mybir.ActivationFunctionType

_CACHE = {}


def _coeff_transform():
    """T27 so that m' = m_flat @ T27.T gives coefficients for the square
    basis [R,G,B,R2,G2,B2,(R+G)^2,(R+B)^2,(G+B)^2]."""
    T9 = np.zeros((9, 9), dtype=np.float64)
    for i in range(3):  # R,G,B linear terms pass through
        T9[i, i] = 1.0
    # new squares: old squares minus half the relevant cross terms
    # old order: 3=r2,4=g2,5=b2,6=rg,7=rb,8=gb
    T9[3, 3] = 1.0
    T9[3, 6] = -0.5
    T9[3, 7] = -0.5
    T9[4, 4] = 1.0
    T9[4, 6] = -0.5
    T9[4, 8] = -0.5
    T9[5, 5] = 1.0
    T9[5, 7] = -0.5
    T9[5, 8] = -0.5
    T9[6, 6] = 0.5  # (R+G)^2 coeff = rg/2
    T9[7, 7] = 0.5
    T9[8, 8] = 0.5
    T27 = np.zeros((27, 27), dtype=np.float64)
    for c in range(3):
        for kn in range(9):
            for ko in range(9):
                T27[3 * kn + c, 3 * ko + c] = T9[kn, ko]
    return T27


def _build():
    nc = bacc.Bacc(
        "TRN2", target_bir_lowering=False, debug=False, num_devices=N_CORES
    )

    x_d = nc.dram_tensor("x_core", [NTILES, P, T * C], F32, kind="ExternalInput")
    hp_d = nc.dram_tensor("h_packed", [P, KT * SPC], F32, kind="ExternalInput")
    w1_d = nc.dram_tensor("w1", [KT, P, HID], F32, kind="ExternalInput")
    b1_d = nc.dram_tensor("b1_rep", [SPC, HID], F32, kind="ExternalInput")
    w2_d = nc.dram_tensor("w2p", [MT, P, MOUT], F32, kind="ExternalInput")
    b2_d = nc.dram_tensor("b2bc", [P, SPC * MOUT], F32, kind="ExternalInput")
    eye_d = nc.dram_tensor("eye2", [SPC, SPC], F32, kind="ExternalInput")
    y_d = nc.dram_tensor("y_core", [NTILES, P, T * C], F32, kind="ExternalOutput")

    with tile.TileContext(nc) as tc:
        with (
            tc.tile_pool(name="mlp", bufs=1) as mlp_pool,
            tc.tile_pool(name="w1s", bufs=3) as w1_pool,
            tc.tile_pool(name="px32", bufs=2) as px32_pool,
            tc.tile_pool(name="pl16", bufs=2) as plane_pool,
            tc.tile_pool(name="ps", bufs=1, space="PSUM") as psum_pool,
        ):
            # ---------------- MLP (TensorE) ----------------
            hp_sb = mlp_pool.tile([P, KT * SPC], F16, tag="hp", name="hp")
            nc.gpsimd.dma_start(out=hp_sb, in_=hp_d[:, :])
            b1_sb = mlp_pool.tile([SPC, HID], F32, tag="b1", name="b1")
            nc.scalar.dma_start(out=b1_sb, in_=b1_d[:, :])
            w2_sb = mlp_pool.tile([P, MT, MOUT], F32, tag="w2", name="w2")
            nc.scalar.dma_start(out=w2_sb, in_=w2_d.rearrange("m p n -> p m n"))
            b2_sb = mlp_pool.tile([P, SPC * MOUT], F32, tag="b2", name="b2")
            nc.scalar.dma_start(out=b2_sb, in_=b2_d[:, :])
            eye_sb = mlp_pool.tile([SPC, SPC], F32, tag="eye", name="eye")
            nc.scalar.dma_start(out=eye_sb, in_=eye_d[:, :])

            # feat = h @ W1: lhsT = h-slices [128, 2] (cheap weight loads),
            # rhs = W1 k-tiles [128, 256] -> psum [2, 256] accumulated.
            feat_ps = psum_pool.tile([SPC, HID], F32, tag="featps", name="featps")
            for kc in range(KT // W1_CH):
                w1_sb = w1_pool.tile([P, W1_CH, HID], F16, tag="w1c", name="w1c")
                nc.gpsimd.dma_start(
                    out=w1_sb,
                    in_=w1_d[kc * W1_CH : (kc + 1) * W1_CH].rearrange(
                        "k p n -> p k n"
                    ),
                )
                for kk in range(W1_CH):
                    k = kc * W1_CH + kk
                    nc.tensor.matmul(
                        feat_ps,
                        hp_sb[:, k * SPC : (k + 1) * SPC],
                        w1_sb[:, kk, :],
                        start=(k == 0),
                        stop=(k == KT - 1),
                    )

            # relu(feat + b1) on DVE (b1 lives on the free dim here)
            feat_sb = mlp_pool.tile([SPC, HID], F32, tag="featsb", name="featsb")
            nc.vector.tensor_add(feat_sb, feat_ps, b1_sb)
            feat_r = mlp_pool.tile([SPC, HID], F32, tag="featr", name="featr")
            nc.vector.tensor_scalar(
                feat_r, feat_sb, 0.0, None, mybir.AluOpType.max
            )

            # transpose feat [2, 256] -> featT tiles [128, 2] via PE
            featT_sb = []
            for mt in range(MT):
                ft_ps = psum_pool.tile(
                    [P, SPC], F32, tag=f"ftps{mt}", name=f"ftps{mt}"
                )
                nc.tensor.transpose(
                    ft_ps, feat_r[:, mt * P : (mt + 1) * P], eye_sb
                )
                ft_sb = mlp_pool.tile(
                    [P, SPC], F32, tag=f"ftsb{mt}", name=f"ftsb{mt}"
                )
                nc.vector.tensor_copy(ft_sb, ft_ps)
                featT_sb.append(ft_sb)

            # fused m-matmul + partition-broadcast: a stride-0 lhsT column
            # makes every output partition compute m[s] = featT[:, s] @ W2'.
            mb_ps = psum_pool.tile([P, SPC * MOUT], F32, tag="mbps", name="mbps")
            for s in range(SPC):
                for mt in range(MT):
                    nc.tensor.matmul(
                        mb_ps[:, s * MOUT : (s + 1) * MOUT],
                        featT_sb[mt][:, s : s + 1].broadcast_to([P, P]),
                        w2_sb[:, mt, :],
                        start=(mt == 0),
                        stop=(mt == MT - 1),
                    )
            mscal = mlp_pool.tile([P, SPC * MOUT], F32, tag="mscal", name="mscal")
            nc.vector.tensor_add(mscal, mb_ps, b2_sb)

            # ---------------- pixel path ----------------
            for t in range(NTILES):
                s = t // TILES_PER_SAMPLE

                def ms(k, c, s=s):
                    j = s * MOUT + 3 * k + c
                    return mscal[:, j : j + 1]

                x32 = px32_pool.tile([P, T, C], F32, tag="x32", name="x32")
                x_dma = nc.sync if t < 2 else nc.gpsimd
                x_dma.dma_start(out=x32, in_=x_d[t].rearrange("p (t c) -> p t c", c=C))

                # deinterleave + cast to fp16 into channel-slices of one
                # wide [P, 3, T] tile (ACT, stride-3 reads)
                rgb = plane_pool.tile([P, C, T], PLDT, tag="rgb", name="rgb")
                nc.scalar.copy(rgb, x32.rearrange("p t c -> p c t"))

                # pair sums (DVE fp16 2x) into a wide tile
                sm = plane_pool.tile([P, C, T], PLDT, tag="sm", name="sm")
                nc.vector.tensor_add(sm[:, 0, :], rgb[:, 0, :], rgb[:, 1, :])
                nc.vector.tensor_add(sm[:, 1, :], rgb[:, 0, :], rgb[:, 2, :])
                nc.vector.tensor_add(sm[:, 2, :], rgb[:, 1, :], rgb[:, 2, :])

                # squares: two wide ACT ops cover all six planes
                sq = plane_pool.tile([P, C, T], PLDT, tag="sq", name="sq")
                qq = plane_pool.tile([P, C, T], PLDT, tag="qq", name="qq")
                nc.scalar.square(sq, rgb)
                nc.scalar.square(qq, sm)

                basis = [
                    rgb[:, 0, :], rgb[:, 1, :], rgb[:, 2, :],
                    sq[:, 0, :], sq[:, 1, :], sq[:, 2, :],
                    qq[:, 0, :], qq[:, 1, :], qq[:, 2, :],
                ]

                y32 = px32_pool.tile([P, T, C], F32, tag="y32", name="y32")
                y32r = y32.rearrange("p t c -> p c t")
                # products: per-channel (distinct scalars) into channel-slices
                # of wide U tiles; adds: channel-merged [P, 3, T] tree.
                # products on ScalarE: 4 per channel for the first half of
                # the tiles, 3 for the rest (balances ACT vs DVE busy time)
                ACT_K = (3, 4, 5, 6) if t < 2 else (3, 4, 5)

                def prods(k, uname):
                    uk = plane_pool.tile(
                        [P, C, T], PLDT, tag=uname, name=f"{uname}_{k}"
                    )
                    for c in range(C):
                        if k in ACT_K:
                            nc.scalar.mul(uk[:, c, :], basis[k], ms(k, c))
                        else:
                            nc.vector.tensor_scalar(
                                uk[:, c, :], basis[k], ms(k, c), None, MULT
                            )
                    return uk

                def tadd(tag, nm, a, b_):
                    o = plane_pool.tile([P, C, T], PLDT, tag=tag, name=nm)
                    nc.vector.tensor_add(o, a, b_)
                    return o

                ua = prods(0, "ua")
                ub = prods(1, "ub")
                ta1 = tadd("ta", f"ta1_{t}", ua, ub)
                ua = prods(2, "ua")
                ub = prods(3, "ub")
                tb1 = tadd("tb", f"tb1_{t}", ua, ub)
                tc1 = tadd("tc", f"tc1_{t}", ta1, tb1)
                ua = prods(4, "ua")
                ub = prods(5, "ub")
                ta2 = tadd("ta", f"ta2_{t}", ua, ub)
                ua = prods(6, "ua")
                ub = prods(7, "ub")
                tb2 = tadd("tb", f"tb2_{t}", ua, ub)
                ta3 = tadd("ta", f"ta3_{t}", ta2, tb2)
                ua = prods(8, "ua")
                tc2 = tadd("tc", f"tc2_{t}", tc1, ua)
                nc.vector.tensor_add(y32r, ta3, tc2)

                nc.sync.dma_start(
                    out=y_d[t].rearrange("p (t c) -> p t c", c=C), in_=y32
                )

    nc.compile()
    return nc


def _prep_inputs(x, histogram, W1, b1, W2, b2):
    """Host-side sharding / layout packing (no arithmetic on data except the
    static linear re-parameterization of the tiny weights W2/b2)."""
    x = np.ascontiguousarray(np.asarray(x, dtype=np.float32))
    hist = np.asarray(histogram, dtype=np.float32).reshape(B, HIST)
    W1 = np.ascontiguousarray(np.asarray(W1, dtype=np.float32))
    b1 = np.asarray(b1, dtype=np.float32)
    W2 = np.asarray(W2, dtype=np.float32)
    b2 = np.asarray(b2, dtype=np.float32)

    T27 = _coeff_transform()
    W2p = np.ascontiguousarray(
        (W2.astype(np.float64) @ T27.T).astype(np.float32).reshape(MT, P, MOUT)
    )
    b2p_flat = (b2.astype(np.float64) @ T27.T).astype(np.float32)

    w1_r = W1.reshape(KT, P, HID)
    b1rep = np.ascontiguousarray(np.broadcast_to(b1, (SPC, HID)))
    b2bc = np.ascontiguousarray(
        np.broadcast_to(np.tile(b2p_flat, SPC), (P, SPC * MOUT))
    )
    eye2 = np.eye(SPC, dtype=np.float32)

    in_maps = []
    for core in range(N_CORES):
        xs = x[core * SPC : (core + 1) * SPC].reshape(-1)
        x_core = np.ascontiguousarray(xs.reshape(NTILES, P, T * C))
        h_core = hist[core * SPC : (core + 1) * SPC]  # [SPC, HIST]
        hp = np.ascontiguousarray(
            h_core.reshape(SPC, KT, P).transpose(2, 1, 0).reshape(P, KT * SPC)
        )
        in_maps.append(
            {
                "x_core": x_core,
                "h_packed": hp,
                "w1": w1_r,
                "b1_rep": b1rep,
                "eye2": eye2,
                "w2p": W2p,
                "b2bc": b2bc,
            }
        )
    return in_maps


def run(trace=False, **inputs):
    if "nc" not in _CACHE:
        _CACHE["nc"] = _build()
    nc = _CACHE["nc"]
    in_maps = _prep_inputs(**inputs)
    res = bass_utils.run_bass_kernel_spmd(
        nc, in_maps, core_ids=list(range(N_CORES)), trace=trace
    )
    outs = np.stack([r["y_core"] for r in res.results])  # [8, NTILES, P, T*C]
    y = outs.reshape(B, H, W, C).astype(np.float32)
    return y, res


def kernel(**inputs) -> np.ndarray:
    y, _ = run(trace=False, **inputs)
    return y


if __name__ == "__main__":
    rng = np.random.default_rng(0)
    ins = {
        "x": rng.random((B, H, W, C), dtype=np.float32),
        "histogram": rng.random((B, 3, 64, 64), dtype=np.float32),
        "W1": (rng.standard_normal((HIST, HID)) / np.sqrt(HIST)).astype(np.float32),
        "b1": np.zeros(HID, np.float32),
        "W2": (rng.standard_normal((HID, MOUT)) / np.sqrt(HID)).astype(np.float32),
        "b2": np.zeros(MOUT, np.float32),
    }
    y = kernel(**ins)
    print("out", y.shape, y.dtype, float(np.abs(y).max()))



# revision 2
# speedup vs baseline: 1.0123x; 1.0123x over previous
"""AWBNet (wo R2) Trainium2 kernel, v2.

Math (per sample b):
  m = reshape(relu(hist_flat @ W1 + b1) @ W2 + b2, [9, 3])
  y[px, c] = m0c r + m1c g + m2c b + m3c r^2 + m4c g^2 + m5c b^2
           + m6c rg + m7c rb + m8c gb

Device strategy (8 cores):
  * MLP: W1 is K-sharded across the 8 cores (1536 rows each). Every core
    computes partial features for ALL 16 samples with 12 efficient M=16
    matmuls, then a ReduceScatter hands each core the finished feature sums
    for its own 2 samples; relu + the tiny W2 matmul run locally.  This cuts
    the per-core W1 stream from 12.6 MB to 1.6 MB and has m ready ~4x
    earlier than streaming the full W1.
  * Pixels: pure data parallel, 2 samples/core, with the two samples split
    across the partition dim (partitions 0-63 = sample 0, 64-127 = sample 1)
    so per-partition scalar operands carry per-sample coefficients.  The
    host pre-packs x into planar R/G/B layout; SWDGE DMAs cast fp32->fp16 in
    flight, so no on-device deinterleave/cast ops are needed.
  * Per-pixel evaluation uses the Horner form
        y_c = R*(a0 + a3 R + a6 G + a7 B) + G*(a1 + a4 G + a8 B)
            + B*(a2 + a5 B)
    = 6 tensor_scalar + 8 tensor_tensor ops per channel, scheduled across
    DVE (tensor_scalar runs in 4x fp16 mode), ACT (scale/bias products) and
    Pool (adds/mults) to balance engine busy time.
  * y is produced in fp16 planes and stored as fp16 (half the write
    traffic); the host casts back to fp32 on assembly.
"""

import sys

import numpy as np

for _p in ("/opt/trn_rl_repo",):
    if _p not in sys.path:
        sys.path.insert(0, _p)

import concourse.bacc as bacc
import concourse.mybir as mybir
import concourse.tile as tile
from concourse import bass_utils

# ---- problem constants (hardcoded per contract) ----
N_CORES = 8
B, H, W, C = 16, 512, 512, 3
SPC = B // N_CORES  # samples per core = 2
PX_SAMPLE = H * W  # 262144
P = 128
LPS = P // SPC  # partition lanes per sample = 64
LANE_PX = PX_SAMPLE // LPS  # 4096 pixels per partition lane
T = 2048  # pixels per partition per tile
NT = LANE_PX // T  # 2 tiles

HIST = 3 * 64 * 64  # 12288
HID = 256
MOUT = 27
KSH = HIST // N_CORES  # 1536 W1 rows per core
KT = KSH // P  # 12 k-tiles per core

F16 = mybir.dt.float16
F32 = mybir.dt.float32
MULT = mybir.AluOpType.mult
ADD = mybir.AluOpType.add

_CACHE = {}


def _build():
    nc = bacc.Bacc(
        "TRN2", target_bir_lowering=False, debug=False, num_devices=N_CORES
    )

    # planar, sample-split-partition pixel input: [tile, ch, part, T]
    x_d = nc.dram_tensor("x_core", [NT, C, P, T], F32, kind="ExternalInput")
    # K-shard of the histogram, packed for lhsT: [128 k2, 12 k1 * 16 s]
    hp_d = nc.dram_tensor("h_packed", [P, KT * B], F32, kind="ExternalInput")
    # K-shard of W1 packed [128 k2, 12 k1, 256 n]
    w1_d = nc.dram_tensor("w1s", [P, KT, HID], F32, kind="ExternalInput")
    b1_d = nc.dram_tensor("b1_rep", [SPC, HID], F32, kind="ExternalInput")
    w2_d = nc.dram_tensor("w2p", [HID // P, P, MOUT], F32, kind="ExternalInput")
    b2_d = nc.dram_tensor("b2per", [P, MOUT], F32, kind="ExternalInput")
    eye_d = nc.dram_tensor("eye2", [SPC, SPC], F32, kind="ExternalInput")
    y_d = nc.dram_tensor("y_core", [NT, C, P, T], F16, kind="ExternalOutput")

    MT = HID // P  # 2

    with tile.TileContext(nc) as tc:
        with (
            tc.tile_pool(name="mlp", bufs=1) as mlp_pool,
            tc.tile_pool(name="dram", bufs=1, space="DRAM") as dram_pool,
            tc.tile_pool(name="xin", bufs=2) as x_pool,
            tc.tile_pool(name="pl", bufs=2) as pl_pool,
            tc.tile_pool(name="yout", bufs=2) as y_pool,
            tc.tile_pool(name="ps", bufs=1, space="PSUM") as psum_pool,
        ):
            # ---------------- MLP (K-sharded + ReduceScatter) ----------------
            hp_sb = mlp_pool.tile([P, KT * B], F16, tag="hp", name="hp")
            nc.gpsimd.dma_start(out=hp_sb, in_=hp_d[:, :])
            w1_sb = mlp_pool.tile([P, KT, HID], F16, tag="w1", name="w1")
            nc.gpsimd.dma_start(out=w1_sb, in_=w1_d[:, :, :])

            b1_sb = mlp_pool.tile([SPC, HID], F32, tag="b1", name="b1")
            nc.scalar.dma_start(out=b1_sb, in_=b1_d[:, :])
            w2_sb = mlp_pool.tile([P, MT, MOUT], F32, tag="w2", name="w2")
            nc.scalar.dma_start(out=w2_sb, in_=w2_d.rearrange("m p n -> p m n"))
            b2_sb = mlp_pool.tile([P, MOUT], F32, tag="b2", name="b2")
            nc.scalar.dma_start(out=b2_sb, in_=b2_d[:, :])
            eye_sb = mlp_pool.tile([SPC, SPC], F32, tag="eye", name="eye")
            nc.scalar.dma_start(out=eye_sb, in_=eye_d[:, :])

            # partial feats for ALL 16 samples from this core's K-shard
            feat_ps = psum_pool.tile([B, HID], F32, tag="featps", name="featps")
            for k1 in range(KT):
                nc.tensor.matmul(
                    feat_ps,
                    hp_sb[:, k1 * B : (k1 + 1) * B],
                    w1_sb[:, k1, :],
                    start=(k1 == 0),
                    stop=(k1 == KT - 1),
                )
            part_sb = mlp_pool.tile([B, HID], F32, tag="part", name="part")
            nc.vector.tensor_copy(part_sb, feat_ps)

            part_dram = dram_pool.tile([B, HID], F32, tag="pdram", name="pdram")
            red_dram = dram_pool.tile([SPC, HID], F32, tag="rdram", name="rdram")
            nc.sync.dma_start(out=part_dram, in_=part_sb)
            nc.gpsimd.collective_compute(
                "ReduceScatter",
                mybir.AluOpType.add,
                replica_groups=[list(range(N_CORES))],
                ins=[part_dram[:, :].opt()],
                outs=[red_dram[:, :].opt()],
            )
            feat_sb = mlp_pool.tile([SPC, HID], F32, tag="feat", name="feat")
            nc.sync.dma_start(out=feat_sb, in_=red_dram)

            # relu(feat + b1)
            feat_b = mlp_pool.tile([SPC, HID], F32, tag="featb", name="featb")
            nc.vector.tensor_add(feat_b, feat_sb, b1_sb)
            feat_r = mlp_pool.tile([SPC, HID], F32, tag="featr", name="featr")
            nc.vector.tensor_scalar(feat_r, feat_b, 0.0, None, mybir.AluOpType.max)

            # transpose feat [2, 256] -> featT tiles [128, 2] via PE
            featT_sb = []
            for mt in range(MT):
                ft_ps = psum_pool.tile([P, SPC], F32, tag=f"ftps{mt}", name=f"ftps{mt}")
                nc.tensor.transpose(ft_ps, feat_r[:, mt * P : (mt + 1) * P], eye_sb)
                ft_sb = mlp_pool.tile([P, SPC], F32, tag=f"ft{mt}", name=f"ft{mt}")
                nc.vector.tensor_copy(ft_sb, ft_ps)
                featT_sb.append(ft_sb)

            # fused m-matmul + partition broadcast (stride-0 lhsT column):
            # mb[p, s*27+j] = m_s[j] for every partition p
            mb_ps = psum_pool.tile([P, SPC * MOUT], F32, tag="mbps", name="mbps")
            for s in range(SPC):
                for mt in range(MT):
                    nc.tensor.matmul(
                        mb_ps[:, s * MOUT : (s + 1) * MOUT],
                        featT_sb[mt][:, s : s + 1].broadcast_to([P, P]),
                        w2_sb[:, mt, :],
                        start=(mt == 0),
                        stop=(mt == MT - 1),
                    )
            # per-partition coefficients: partitions 0-63 <- sample 0,
            # 64-127 <- sample 1 (plus bias b2)
            mper = mlp_pool.tile([P, MOUT], F32, tag="mper", name="mper")
            for s in range(SPC):
                lo, hi = s * LPS, (s + 1) * LPS
                nc.vector.tensor_add(
                    mper[lo:hi, :],
                    mb_ps[lo:hi, s * MOUT : (s + 1) * MOUT],
                    b2_sb[lo:hi, :],
                )

            def ms(k, c):
                return mper[:, 3 * k + c : 3 * k + c + 1]

            # ---------------- pixel path (Horner) ----------------
            for t in range(NT):
                xt = x_pool.tile([P, C, T], F16, tag="xt", name=f"xt{t}")
                nc.gpsimd.dma_start(
                    out=xt, in_=x_d[t].rearrange("c p j -> p c j")
                )
                R, G, Bp = xt[:, 0, :], xt[:, 1, :], xt[:, 2, :]

                ysb = y_pool.tile([P, C, T], F16, tag="ysb", name=f"ysb{t}")

                def pt(tag, nm):
                    return pl_pool.tile([P, T], F16, tag=tag, name=nm)

                for c in range(C):
                    sfx = f"_{t}{c}"
                    # --- products (tensor_scalar / ACT affine) ---
                    a1 = pt("a1", "a1" + sfx)  # a3*R + a0
                    nc.vector.tensor_scalar(a1, R, ms(3, c), ms(0, c), MULT, ADD)
                    a2 = pt("a2", "a2" + sfx)  # a6*G
                    nc.scalar.mul(a2, G, ms(6, c))
                    a3 = pt("a3", "a3" + sfx)  # a7*B
                    nc.scalar.mul(a3, Bp, ms(7, c))
                    b1t = pt("b1t", "b1t" + sfx)  # a4*G + a1
                    nc.vector.tensor_scalar(b1t, G, ms(4, c), ms(1, c), MULT, ADD)
                    b2t = pt("b2t", "b2t" + sfx)  # a8*B
                    nc.scalar.mul(b2t, Bp, ms(8, c))
                    cc = pt("cc", "cc" + sfx)  # a5*B + a2
                    nc.scalar.activation(
                        cc, Bp, mybir.ActivationFunctionType.Identity,
                        bias=ms(2, c), scale=ms(5, c),
                    )
                    # --- combine ---
                    a12 = pt("a12", "a12" + sfx)
                    nc.vector.tensor_add(a12, a1, a2)
                    aa = pt("aa", "aa" + sfx)
                    nc.vector.tensor_add(aa, a12, a3)
                    bb = pt("bb", "bb" + sfx)
                    nc.gpsimd.tensor_add(bb, b1t, b2t)
                    ra = pt("ra", "ra" + sfx)
                    nc.vector.tensor_mul(ra, R, aa)
                    gb = pt("gb", "gb" + sfx)
                    nc.vector.tensor_mul(gb, G, bb)
                    bc = pt("bc", "bc" + sfx)
                    nc.gpsimd.tensor_mul(bc, Bp, cc)
                    y1 = pt("y1", "y1" + sfx)
                    nc.vector.tensor_add(y1, ra, gb)
                    nc.vector.tensor_add(ysb[:, c, :], y1, bc)

                nc.sync.dma_start(
                    out=y_d[t].rearrange("c p j -> p c j"), in_=ysb
                )

    nc.compile()
    return nc


def _prep_inputs(x, histogram, W1, b1, W2, b2):
    """Host-side sharding / layout packing (pure data movement; the only
    dtype change is fp32 passthrough — device DMAs do the fp16 casts)."""
    x = np.asarray(x, dtype=np.float32)
    hist = np.asarray(histogram, dtype=np.float32).reshape(B, HIST)
    W1 = np.asarray(W1, dtype=np.float32)
    b1 = np.asarray(b1, dtype=np.float32)
    W2 = np.asarray(W2, dtype=np.float32)
    b2 = np.asarray(b2, dtype=np.float32)

    w2p = np.ascontiguousarray(W2.reshape(HID // P, P, MOUT))
    b1rep = np.ascontiguousarray(np.broadcast_to(b1, (SPC, HID)))
    b2per = np.ascontiguousarray(np.broadcast_to(b2, (P, MOUT)))
    eye2 = np.eye(SPC, dtype=np.float32)

    in_maps = []
    for core in range(N_CORES):
        # pixels: [s, px, ch] -> [t, ch, (s,l), j] with px = (2l + t)*T + j
        xs = x[core * SPC : (core + 1) * SPC].reshape(SPC, LPS, NT, T, C)
        x_core = np.ascontiguousarray(
            xs.transpose(2, 4, 0, 1, 3).reshape(NT, C, P, T)
        )
        # histogram K-shard, packed [k2, k1*16 + s]
        hs = hist[:, core * KSH : (core + 1) * KSH]  # [16, 1536]
        hp = np.ascontiguousarray(
            hs.reshape(B, KT, P).transpose(2, 1, 0).reshape(P, KT * B)
        )
        # W1 K-shard packed [k2, k1, n]
        w1s = W1[core * KSH : (core + 1) * KSH]  # [1536, 256]
        w1p = np.ascontiguousarray(w1s.reshape(KT, P, HID).transpose(1, 0, 2))
        in_maps.append(
            {
                "x_core": x_core,
                "h_packed": hp,
                "w1s": w1p,
                "b1_rep": b1rep,
                "w2p": w2p,
                "b2per": b2per,
                "eye2": eye2,
            }
        )
    return in_maps


def run(trace=False, **inputs):
    if "nc" not in _CACHE:
        _CACHE["nc"] = _build()
    nc = _CACHE["nc"]
    in_maps = _prep_inputs(**inputs)
    res = bass_utils.run_bass_kernel_spmd(
        nc, in_maps, core_ids=list(range(N_CORES)), trace=trace
    )
    outs = np.stack([r["y_core"] for r in res.results])  # [8, NT, C, P, T] f16
    # [core, t, c, (s,l), j] -> [core, s, l, t, j, c] -> [B, H, W, C]
    y = (
        outs.reshape(N_CORES, NT, C, SPC, LPS, T)
        .transpose(0, 3, 4, 1, 5, 2)
        .reshape(B, H, W, C)
        .astype(np.float32)
    )
    return y, res


def kernel(**inputs) -> np.ndarray:
    y, _ = run(trace=False, **inputs)
    return y


if __name__ == "__main__":
    rng = np.random.default_rng(0)
    ins = {
        "x": rng.random((B, H, W, C), dtype=np.float32),
        "histogram": rng.random((B, 3, 64, 64), dtype=np.float32),
        "W1": (rng.standard_normal((HIST, HID)) / np.sqrt(HIST)).astype(np.float32),
        "b1": np.zeros(HID, np.float32),
        "W2": (rng.standard_normal((HID, MOUT)) / np.sqrt(HID)).astype(np.float32),
        "b2": np.zeros(MOUT, np.float32),
    }
    y = kernel(**ins)
    print("out", y.shape, y.dtype, float(np.abs(y).max()))


# revision 3
# speedup vs baseline: 1.2953x; 1.2795x over previous
"""AWBNet (wo R2) Trainium2 kernel, v2.

Math (per sample b):
  m = reshape(relu(hist_flat @ W1 + b1) @ W2 + b2, [9, 3])
  y[px, c] = m0c r + m1c g + m2c b + m3c r^2 + m4c g^2 + m5c b^2
           + m6c rg + m7c rb + m8c gb

Device strategy (8 cores, pure data parallel, 2 samples/core):
  * MLP: the full W1 is streamed per core as fp16 (host-cast; the device
    DMA would cast to fp16 anyway, this just halves the HBM read) on the
    sync HWDGE ring, in chunks pipelined with the 96 accumulating PE
    matmuls (lhsT = packed histogram slices [128, 2]).  feat -> relu ->
    PE transpose -> stride-0-broadcast W2 matmul produce mscal[P, 54]
    fp32 coefficients replicated across partitions.
  * Pixels: one tile per sample, [128, 2048] planar fp16 planes loaded
    via SWDGE cast DMAs (host pre-packs x planar, so there is no on-device
    deinterleave).  Per-pixel evaluation uses the Horner form
        y_c = R*(a0 + a3 R + a6 G + a7 B) + G*(a1 + a4 G + a8 B)
            + B*(a2 + a5 B)
    with the per-channel scalar products on ACT (scale/bias activations)
    and DVE (4x-mode tensor_scalar), and all tensor-tensor combines as
    channel-merged wide [128, 3, 2048] DVE ops (2x fp16 mode).  The Pool
    engine is deliberately compute-free: its Q7 tensor ops are slow and
    degrade concurrent DVE throughput (measured), so it only issues the
    SWDGE cast DMAs.
  * y is produced as fp16 planes and stored fp16 (half the write
    traffic); the host casts back to fp32 on assembly.
"""

import sys

import numpy as np

for _p in ("/opt/trn_rl_repo",):
    if _p not in sys.path:
        sys.path.insert(0, _p)

import concourse.bacc as bacc
import concourse.mybir as mybir
import concourse.tile as tile
from concourse import bass_utils

# ---- problem constants (hardcoded per contract) ----
N_CORES = 8
B, H, W, C = 16, 512, 512, 3
SPC = B // N_CORES  # samples per core = 2
PX_SAMPLE = H * W  # 262144
P = 128
T = PX_SAMPLE // P  # 2048 pixels per partition; one tile per sample
NT = SPC  # 2 tiles per core

HIST = 3 * 64 * 64  # 12288
HID = 256
MOUT = 27
KT = HIST // P  # 96 k-tiles
W1_CH = 8  # k-tiles per W1 chunk DMA
NCH = KT // W1_CH  # 12 chunks

F16 = mybir.dt.float16
F32 = mybir.dt.float32
MULT = mybir.AluOpType.mult
ADD = mybir.AluOpType.add
AF = mybir.ActivationFunctionType

_CACHE = {}


def _build():
    nc = bacc.Bacc(
        "TRN2", target_bir_lowering=False, debug=False, num_devices=N_CORES
    )

    # planar pixel input [tile(=sample), ch, part, T]
    x_d = nc.dram_tensor("x_core", [NT, C, P, T], F32, kind="ExternalInput")
    # histogram for this core's 2 samples, packed [k2, k1*2 + s]
    hp_d = nc.dram_tensor("h_packed", [P, KT * SPC], F32, kind="ExternalInput")
    # full W1, host-cast fp16, packed [k2, k1, n]
    w1_d = nc.dram_tensor("w1h", [P, KT, HID], F16, kind="ExternalInput")
    b1_d = nc.dram_tensor("b1_rep", [SPC, HID], F32, kind="ExternalInput")
    w2_d = nc.dram_tensor("w2p", [HID // P, P, MOUT], F32, kind="ExternalInput")
    b2_d = nc.dram_tensor("b2bc", [P, SPC * MOUT], F32, kind="ExternalInput")
    eye_d = nc.dram_tensor("eye2", [SPC, SPC], F32, kind="ExternalInput")
    y_d = nc.dram_tensor("y_core", [NT, C, P, T], F16, kind="ExternalOutput")

    MT = HID // P  # 2

    with tile.TileContext(nc) as tc:
        with (
            tc.tile_pool(name="mlp", bufs=1) as mlp_pool,
            tc.tile_pool(name="w1s", bufs=2) as w1_pool,
            tc.tile_pool(name="xin", bufs=2) as x_pool,
            tc.tile_pool(name="pla", bufs=2) as pa_pool,
            tc.tile_pool(name="plb", bufs=1) as pb_pool,
            tc.tile_pool(name="yout", bufs=1) as y_pool,
            tc.tile_pool(name="ps", bufs=1, space="PSUM") as psum_pool,
        ):
            # ---------------- MLP ----------------
            hp_sb = mlp_pool.tile([P, KT * SPC], F16, tag="hp", name="hp")
            nc.gpsimd.dma_start(out=hp_sb, in_=hp_d[:, :])

            b1_sb = mlp_pool.tile([SPC, HID], F32, tag="b1", name="b1")
            nc.scalar.dma_start(out=b1_sb, in_=b1_d[:, :])
            w2_sb = mlp_pool.tile([P, MT, MOUT], F32, tag="w2", name="w2")
            nc.scalar.dma_start(out=w2_sb, in_=w2_d.rearrange("m p n -> p m n"))
            b2_sb = mlp_pool.tile([P, SPC * MOUT], F32, tag="b2", name="b2")
            nc.scalar.dma_start(out=b2_sb, in_=b2_d[:, :])
            eye_sb = mlp_pool.tile([SPC, SPC], F32, tag="eye", name="eye")
            nc.scalar.dma_start(out=eye_sb, in_=eye_d[:, :])

            feat_ps = psum_pool.tile([SPC, HID], F32, tag="featps", name="featps")
            for ci in range(NCH):
                w1c = w1_pool.tile([P, W1_CH, HID], F16, tag="w1c", name=f"w1c{ci}")
                nc.sync.dma_start(
                    out=w1c, in_=w1_d[:, ci * W1_CH : (ci + 1) * W1_CH, :]
                )
                for kk in range(W1_CH):
                    k = ci * W1_CH + kk
                    nc.tensor.matmul(
                        feat_ps,
                        hp_sb[:, k * SPC : (k + 1) * SPC],
                        w1c[:, kk, :],
                        start=(k == 0),
                        stop=(k == KT - 1),
                    )

            feat_b = mlp_pool.tile([SPC, HID], F32, tag="featb", name="featb")
            nc.vector.tensor_add(feat_b, feat_ps, b1_sb)
            feat_r = mlp_pool.tile([SPC, HID], F32, tag="featr", name="featr")
            nc.vector.tensor_scalar(feat_r, feat_b, 0.0, None, mybir.AluOpType.max)

            featT_sb = []
            for mt in range(MT):
                ft_ps = psum_pool.tile([P, SPC], F32, tag=f"ftps{mt}", name=f"ftps{mt}")
                nc.tensor.transpose(ft_ps, feat_r[:, mt * P : (mt + 1) * P], eye_sb)
                ft_sb = mlp_pool.tile([P, SPC], F32, tag=f"ft{mt}", name=f"ft{mt}")
                nc.vector.tensor_copy(ft_sb, ft_ps)
                featT_sb.append(ft_sb)

            mb_ps = psum_pool.tile([P, SPC * MOUT], F32, tag="mbps", name="mbps")
            for s in range(SPC):
                for mt in range(MT):
                    nc.tensor.matmul(
                        mb_ps[:, s * MOUT : (s + 1) * MOUT],
                        featT_sb[mt][:, s : s + 1].broadcast_to([P, P]),
                        w2_sb[:, mt, :],
                        start=(mt == 0),
                        stop=(mt == MT - 1),
                    )
            mscal = mlp_pool.tile([P, SPC * MOUT], F32, tag="mscal", name="mscal")
            nc.vector.tensor_add(mscal, mb_ps, b2_sb)

            # ---------------- pixel path (Horner) ----------------
            for t in range(NT):
                def ms(k, c, s=t):
                    j = s * MOUT + 3 * k + c
                    return mscal[:, j : j + 1]

                xt = x_pool.tile([P, C, T], F16, tag="xt", name=f"xt{t}")
                nc.gpsimd.dma_start(out=xt, in_=x_d[t].rearrange("c p j -> p c j"))
                R, G, Bp = xt[:, 0, :], xt[:, 1, :], xt[:, 2, :]
                Rw = xt[:, 0:1, :].broadcast_to([P, C, T])
                Gw = xt[:, 1:2, :].broadcast_to([P, C, T])
                Bw = xt[:, 2:3, :].broadcast_to([P, C, T])

                ysb = y_pool.tile([P, C, T], F16, tag="ysb", name=f"ysb{t}")

                # per-channel scalar products into channel slices of wide tiles
                a1w = pa_pool.tile([P, C, T], F16, tag="a1w", name=f"a1w{t}")
                a2w = pa_pool.tile([P, C, T], F16, tag="a2w", name=f"a2w{t}")
                a3w = pa_pool.tile([P, C, T], F16, tag="a3w", name=f"a3w{t}")
                b1w = pa_pool.tile([P, C, T], F16, tag="b1w", name=f"b1w{t}")
                b2w = pb_pool.tile([P, C, T], F16, tag="b2w", name=f"b2w{t}")
                ccw = pb_pool.tile([P, C, T], F16, tag="ccw", name=f"ccw{t}")
                for c in range(C):
                    # DVE 4x tensor_scalar: a3*R + a0
                    nc.vector.tensor_scalar(
                        a1w[:, c, :], R, ms(3, c), ms(0, c), MULT, ADD
                    )
                    # ACT products
                    nc.scalar.mul(a2w[:, c, :], G, ms(6, c))
                    nc.scalar.mul(a3w[:, c, :], Bp, ms(7, c))
                    nc.scalar.activation(
                        b1w[:, c, :], G, AF.Identity, bias=ms(1, c), scale=ms(4, c)
                    )
                    nc.scalar.mul(b2w[:, c, :], Bp, ms(8, c))
                    nc.scalar.activation(
                        ccw[:, c, :], Bp, AF.Identity, bias=ms(2, c), scale=ms(5, c)
                    )

                # wide channel-merged combines on DVE
                a12 = pb_pool.tile([P, C, T], F16, tag="a12", name=f"a12{t}")
                nc.vector.tensor_add(a12, a1w, a2w)
                aa = pb_pool.tile([P, C, T], F16, tag="aa", name=f"aa{t}")
                nc.vector.tensor_add(aa, a12, a3w)
                bb = pb_pool.tile([P, C, T], F16, tag="bb", name=f"bb{t}")
                nc.vector.tensor_add(bb, b1w, b2w)
                ra = pa_pool.tile([P, C, T], F16, tag="a2w", name=f"ra{t}")
                nc.vector.tensor_mul(ra, Rw, aa)
                gb = pa_pool.tile([P, C, T], F16, tag="a3w", name=f"gb{t}")
                nc.vector.tensor_mul(gb, Gw, bb)
                bc = pa_pool.tile([P, C, T], F16, tag="a1w", name=f"bc{t}")
                nc.vector.tensor_mul(bc, Bw, ccw)
                y1 = pa_pool.tile([P, C, T], F16, tag="b1w", name=f"y1_{t}")
                nc.vector.tensor_add(y1, ra, gb)
                nc.vector.tensor_add(ysb, y1, bc)

                nc.sync.dma_start(out=y_d[t].rearrange("c p j -> p c j"), in_=ysb)

    nc.compile()
    return nc


def _prep_inputs(x, histogram, W1, b1, W2, b2):
    """Host-side sharding / layout packing.  The only host dtype change is
    W1 fp32->fp16 (identical values to what the device cast DMA would
    produce; halves the streamed bytes)."""
    x = np.asarray(x, dtype=np.float32)
    hist = np.asarray(histogram, dtype=np.float32).reshape(B, HIST)
    W1 = np.asarray(W1, dtype=np.float32)
    b1 = np.asarray(b1, dtype=np.float32)
    W2 = np.asarray(W2, dtype=np.float32)
    b2 = np.asarray(b2, dtype=np.float32)

    # [k, n] -> [k2, k1, n] fp16
    w1h = np.ascontiguousarray(
        W1.reshape(KT, P, HID).transpose(1, 0, 2).astype(np.float16)
    )
    w2p = np.ascontiguousarray(W2.reshape(HID // P, P, MOUT))
    b1rep = np.ascontiguousarray(np.broadcast_to(b1, (SPC, HID)))
    b2bc = np.ascontiguousarray(np.broadcast_to(np.tile(b2, SPC), (P, SPC * MOUT)))
    eye2 = np.eye(SPC, dtype=np.float32)

    in_maps = []
    for core in range(N_CORES):
        # pixels of sample s: [px, ch] -> [ch, p, j], px = p*T + j
        xs = x[core * SPC : (core + 1) * SPC].reshape(SPC, P, T, C)
        x_core = np.ascontiguousarray(xs.transpose(0, 3, 1, 2))
        hs = hist[core * SPC : (core + 1) * SPC]  # [2, HIST]
        hp = np.ascontiguousarray(
            hs.reshape(SPC, KT, P).transpose(2, 1, 0).reshape(P, KT * SPC)
        )
        in_maps.append(
            {
                "x_core": x_core,
                "h_packed": hp,
                "w1h": w1h,
                "b1_rep": b1rep,
                "w2p": w2p,
                "b2bc": b2bc,
                "eye2": eye2,
            }
        )
    return in_maps


def run(trace=False, **inputs):
    if "nc" not in _CACHE:
        _CACHE["nc"] = _build()
    nc = _CACHE["nc"]
    in_maps = _prep_inputs(**inputs)
    res = bass_utils.run_bass_kernel_spmd(
        nc, in_maps, core_ids=list(range(N_CORES)), trace=trace
    )
    outs = np.stack([r["y_core"] for r in res.results])  # [8, NT, C, P, T] f16
    # [core, s, c, p, j] -> [B, H, W, C]
    y = (
        outs.reshape(N_CORES * SPC, C, P * T)
        .transpose(0, 2, 1)
        .reshape(B, H, W, C)
        .astype(np.float32)
    )
    return y, res


def kernel(**inputs) -> np.ndarray:
    y, _ = run(trace=False, **inputs)
    return y


if __name__ == "__main__":
    rng = np.random.default_rng(0)
    ins = {
        "x": rng.random((B, H, W, C), dtype=np.float32),
        "histogram": rng.random((B, 3, 64, 64), dtype=np.float32),
        "W1": (rng.standard_normal((HIST, HID)) / np.sqrt(HIST)).astype(np.float32),
        "b1": np.zeros(HID, np.float32),
        "W2": (rng.standard_normal((HID, MOUT)) / np.sqrt(HID)).astype(np.float32),
        "b2": np.zeros(MOUT, np.float32),
    }
    y = kernel(**ins)
    print("out", y.shape, y.dtype, float(np.abs(y).max()))
